# revision 1
# baseline (speedup 1.0000x reference)
"""OTAM (5-way 5-shot video few-shot) kernel for Trainium2, 8 NeuronCores.

Self-contained: kernel(**inputs) takes full inputs, shards 512 queries over
8 cores (64 each), runs a Bass/Tile kernel per core, gathers class means.
"""
import sys
sys.path.insert(0, "/opt/trn_rl_repo")
import numpy as np
from contextlib import ExitStack

import concourse.bacc as bacc
import concourse.tile as tile
from concourse import mybir

F32 = mybir.dt.float32
F32R = mybir.dt.float32r
I32 = mybir.dt.int32
AF = mybir.ActivationFunctionType
ALU = mybir.AluOpType
LN2 = float(np.log(2.0))

NS, T, D = 25, 16, 2048
NQ_CORE = 64
G = NQ_CORE // 8
NSTAU = NS * T              # 400
KCH = D // 128              # 16
SROWS = [128, 128, 128, 16]
WAVE = 4                    # query groups per ACT-coherent wave


def build_core_kernel():
    nc = bacc.Bacc("TRN2", target_bir_lowering=False, debug=False)

    q_d = nc.dram_tensor("q", [NQ_CORE * T, D], F32, kind="ExternalInput").ap()
    s_d = nc.dram_tensor("s", [NSTAU, D], F32, kind="ExternalInput").ap()
    eye_d = nc.dram_tensor("eye", [128, 128], F32, kind="ExternalInput").ap()
    out_d = nc.dram_tensor("out", [128, NS], F32, kind="ExternalOutput").ap()

    with tile.TileContext(nc) as tc, ExitStack() as ctx:
        const = ctx.enter_context(tc.tile_pool(name="const", bufs=1))
        eye = const.tile([128, 128], F32, tag="eye")
        nc.sync.dma_start(out=eye[:], in_=eye_d)
        bias_m10 = const.tile([128, 1], F32, tag="bias_m10")
        nc.vector.memset(bias_m10[:], -10.0)

        stp = ctx.enter_context(tc.tile_pool(name="stp", bufs=1))
        st_r = stp.tile([128, KCH, NSTAU], F32R, tag="st_r")

        psp = ctx.enter_context(tc.tile_pool(name="psp", bufs=2, space="PSUM"))
        pst = ctx.enter_context(tc.tile_pool(name="pst", bufs=2, space="PSUM"))

        dmp = ctx.enter_context(tc.tile_pool(name="dmp", bufs=1))
        nsc = ctx.enter_context(tc.tile_pool(name="nsc", bufs=1))

        def rownorms(x, nrow, scale, tag):
            """[128,1] tile = (scale * sum(x^2))^(-1/2) on rows 0:nrow (ACT)."""
            sq = nsc.tile([128, 1], F32, tag=tag + "_sq")
            dump = dmp.tile([128, D], F32, tag="normdump")
            nc.scalar.activation(dump[:nrow], x[:nrow], AF.Square,
                                 accum_out=sq[:nrow])
            rs = nsc.tile([128, 1], F32, tag=tag + "_rs")
            nc.scalar.activation(rs[:nrow], sq[:nrow], AF.Abs_reciprocal_sqrt,
                                 scale=scale)
            return rs

        # ---------------- S phase ----------------
        with tc.tile_pool(name="snatp", bufs=1) as snatp:
            snat = []
            for i, nrow in enumerate(SROWS):
                t_ = snatp.tile([128, D], F32, tag=f"snat{i}")
                nc.sync.dma_start(out=t_[:nrow], in_=s_d[128 * i:128 * i + nrow, :])
                rs = rownorms(t_, nrow, 1.0, f"sn{i}")
                nc.vector.tensor_scalar(t_[:nrow], t_[:nrow], rs[:nrow], None,
                                        op0=ALU.mult)
                snat.append(t_)
            for k in range(KCH):
                ps = pst.tile([128, 512], F32, tag="tps")
                for i, nrow in enumerate(SROWS):
                    nc.tensor.transpose(ps[:, 128 * i:128 * i + nrow],
                                        snat[i][:nrow, 128 * k:128 * (k + 1)],
                                        eye[:nrow, :nrow])
                nc.scalar.copy(st_r[:, k, :], ps[:, 0:NSTAU])

        # ---------------- C tensors ----------------
        cp = ctx.enter_context(tc.tile_pool(name="cp", bufs=1))
        c_t = cp.tile([128, NS, T, T], F32, tag="c_t")       # [p][s][m'][l]
        # staging: partitions 0:64 = araw (pin,c,l) 8192; 64:128 = braw (m',s,l) 6400
        stg = cp.tile([128, 128 * 4 * T], F32, tag="stg")

        qtp = ctx.enter_context(tc.tile_pool(name="qtp", bufs=1))
        qt_r = qtp.tile([128, KCH, NQ_CORE * T], F32R, tag="qt_r")

        qnp = ctx.enter_context(tc.tile_pool(name="qnp", bufs=1))
        t1p = ctx.enter_context(tc.tile_pool(name="t1p", bufs=2))
        t1tp = ctx.enter_context(tc.tile_pool(name="t1tp", bufs=2))

        # ---------------- Q phase: 2 waves of 4 groups ----------------
        for wv in range(G // WAVE):
            qns, rqs = [], []
            for gi in range(WAVE):
                g = wv * WAVE + gi
                qn = qnp.tile([128, D], F32, tag=f"qnat{gi}")
                nc.sync.dma_start(out=qn[:], in_=q_d[128 * g:128 * (g + 1), :])
                qns.append(qn)
            for gi in range(WAVE):
                rqs.append(rownorms(qns[gi], 128, 0.01, f"rq{gi}"))
            for gi in range(WAVE):
                g = wv * WAVE + gi
                qn, rq10 = qns[gi], rqs[gi]
                for c in range(2):
                    ps = psp.tile([128, 1024], F32, tag="q_ps")
                    for j in range(8):
                        k = 8 * c + j
                        nc.tensor.transpose(ps[:, 128 * j:128 * (j + 1)],
                                            qn[:, 128 * k:128 * (k + 1)], eye[:])
                    nc.scalar.copy(
                        qt_r[:, 8 * c:8 * c + 8, 128 * g:128 * (g + 1)],
                        ps[:].rearrange("p (j f) -> p j f", j=8))
                mm = psp.tile([128, NSTAU], F32, tag="mm_ps")
                for k in range(KCH):
                    nc.tensor.matmul(mm[:], qt_r[:, k, 128 * g:128 * (g + 1)],
                                     st_r[:, k, :],
                                     start=(k == 0), stop=(k == KCH - 1))
                t1 = t1p.tile([128, NSTAU], F32, tag="t1")
                nc.scalar.activation(t1[:], mm[:], AF.Exp, bias=bias_m10[:],
                                     scale=rq10[:])
                for qi in range(8):
                    out_b = stg[64 + 8 * g + qi: 64 + 8 * g + qi + 1, 0:6400] \
                        .rearrange("one (m f) -> one m f", m=16)
                    nc.scalar.dma_start(out=out_b, in_=t1[16 * qi:16 * qi + 16, :])
                ps2 = pst.tile([128, 512], F32, tag="tps")
                for c in range(4):
                    w = min(128, NSTAU - 128 * c)
                    nc.tensor.transpose(ps2[:w, 128 * c:128 * c + 128],
                                        t1[:, 128 * c:128 * c + w], eye[:])
                t1t = t1tp.tile([128, 512], F32, tag="t1t")
                nc.gpsimd.memset(t1t[:, 384:512], 0.0)
                nc.vector.tensor_copy(t1t[:, 0:384], ps2[:, 0:384])
                nc.vector.tensor_copy(t1t[0:16, 384:512], ps2[0:16, 384:512])
                for qi in range(8):
                    out_a = stg[8 * g + qi: 8 * g + qi + 1, :]
                    in_a = t1t.rearrange("p (c q l) -> p c q l", c=4, q=8)[:, :, qi, :]
                    nc.scalar.dma_start(out=out_a, in_=in_a)

        nc.gpsimd.tensor_copy(
            c_t[64:128],
            stg[64:128, 0:6400].rearrange("p (m s l) -> p s m l", m=16, s=NS))
        # araw[q][pin][c][l] -> c_t[q] flat (s m l) with (s*16+m') = c*128+pin
        caflat = c_t[0:64].rearrange("p s m l -> p (s m) l")
        av = stg[0:64, :].rearrange("p (pin c l) -> p pin c l", pin=128, c=4)
        nc.vector.tensor_copy(
            caflat[:, 0:384, :].rearrange("p (c pin) l -> p pin c l", c=3),
            av[:, :, 0:3, :])
        nc.vector.tensor_copy(caflat[:, 384:400, :], av[:, 0:16, 3, :])

        # ---------------- DP phase (exp domain) ----------------
        dpp = ctx.enter_context(tc.tile_pool(name="dpp", bufs=1))
        w_t = dpp.tile([128, NS, T + 1], F32, tag="w_t")
        nc.vector.memset(w_t[:], 2.0)
        nc.vector.memset(w_t[:, :, 0:1], 1.0)
        o_t = dpp.tile([128, NS], F32, tag="o_t")
        nc.vector.memset(o_t[:], 0.0)
        scratch = dpp.tile([128, NS, T], F32, tag="scratch")
        kmax = dpp.tile([128, NS], F32, tag="kmax")
        masked = dpp.tile([128, NS], I32, tag="masked")
        krec = dpp.tile([128, NS], I32, tag="krec")
        ef = dpp.tile([128, NS], F32, tag="ef")
        otmp = dpp.tile([128, NS], F32, tag="otmp")

        def renorm(a):
            wsl = w_t[:, :, a:T + 1]
            nc.vector.tensor_reduce(kmax[:], wsl, axis=mybir.AxisListType.X,
                                    op=ALU.max)
            nc.vector.tensor_scalar(masked[:], kmax[:].bitcast(I32),
                                    0x7F800000, None, op0=ALU.bitwise_and)
            nc.vector.tensor_scalar(krec[:], masked[:], 0x7F000000, -1,
                                    op0=ALU.subtract, op1=ALU.mult)
            nc.vector.tensor_copy(ef[:], masked[:])
            nc.vector.tensor_scalar(otmp[:], ef[:], LN2 / (1 << 23),
                                    -127.0 * LN2, op0=ALU.mult, op1=ALU.add)
            nc.vector.tensor_tensor(o_t[:], o_t[:], otmp[:], op=ALU.add)
            nc.vector.tensor_tensor(
                wsl, wsl,
                krec[:].bitcast(F32).unsqueeze(-1).broadcast_to((128, NS, T + 1 - a)),
                op=ALU.mult)

        for m in range(2, T + 3):           # m = 2..18
            j0 = max(1, m - 2)
            wm = (T + 1) - j0
            if m == T + 2:                  # last: dup, cost=1, in-place
                nc.vector.scalar_tensor_tensor(w_t[:, :, T:T + 1],
                                               w_t[:, :, T:T + 1], 2.0,
                                               w_t[:, :, T - 1:T],
                                               op0=ALU.mult, op1=ALU.add)
                break
            wact = w_t[:, :, j0:T + 1]
            wsh = w_t[:, :, j0 - 1:T]
            tmp = scratch[:, :, 0:wm]
            if m == 2:
                nc.vector.scalar_tensor_tensor(tmp, wact, 2.0, wsh,
                                               op0=ALU.mult, op1=ALU.add)
            else:
                nc.vector.tensor_tensor(tmp, wact, wsh, op=ALU.add)
            cs = c_t[:, :, m - 2, j0 - 1: j0 - 1 + wm]
            nc.vector.tensor_tensor(wact, tmp, cs, op=ALU.mult)
            if m in (5, 9, 13, 16):
                renorm(m - 2)

        lnw = dpp.tile([128, NS], F32, tag="lnw")
        nc.scalar.activation(lnw[:], w_t[:, :, T], AF.Ln)
        ans = dpp.tile([128, NS], F32, tag="ans")
        nc.vector.tensor_tensor(ans[:], lnw[:], o_t[:], op=ALU.add)
        nc.sync.dma_start(out=out_d, in_=ans[:])

    nc.compile()
    return nc


_NC_CACHE = {}


def _get_nc():
    if "nc" not in _NC_CACHE:
        _NC_CACHE["nc"] = build_core_kernel()
    return _NC_CACHE["nc"]


def kernel(support_features, target_features, support_labels):
    out, _ = host_kernel(support_features, target_features, support_labels,
                         nc=_get_nc())
    return out


def host_kernel(support_features, target_features, support_labels, nc=None,
                run_hw=True, trace=False):
    n_support, T_, d = support_features.shape
    nq = target_features.shape[0]
    assert (n_support, T_, d) == (NS, T, D) and nq == 512
    if nc is None:
        nc = build_core_kernel()
    eye = np.eye(128, dtype=np.float32)
    s_flat = np.ascontiguousarray(support_features.reshape(NSTAU, D))
    in_maps = []
    for c in range(8):
        qs = target_features[64 * c:64 * (c + 1)].reshape(NQ_CORE * T, D)
        in_maps.append({"q": np.ascontiguousarray(qs), "s": s_flat, "eye": eye})
    from concourse.bass_utils import run_bass_kernel_spmd
    res = run_bass_kernel_spmd(nc, in_maps, list(range(8)), trace=trace)
    outs = [np.asarray(r["out"]) for r in res.results]
    dists = np.concatenate([-0.1 * (o[0:64] + o[64:128]) for o in outs], axis=0)
    onehot = (np.asarray(support_labels)[:, None]
              == np.arange(5)[None, :]).astype(np.float32)
    class_dists = (dists.astype(np.float32) @ onehot) / onehot.sum(axis=0)
    return class_dists.astype(np.float32), res



# revision 6
# speedup vs baseline: 1.8786x; 1.8786x over previous
"""OTAM (5-way 5-shot video few-shot) kernel for Trainium2, 8 NeuronCores.

Self-contained: kernel(**inputs) takes full inputs, shards 512 queries over
8 cores (64 each), runs a Bass/Tile kernel per core, gathers class means.

v1 design (vs v0 baseline at 328us):
 - bf16 matmuls + transposes (tolerance 2e-2 >> bf16 error)
 - fold q-norm into the exp scale (no q normalize pass); s normalized once
 - norms via ACT Square+accum + Quake rsqrt on DVE (no AbsRsqrt table;
   tensor_tensor_reduce crashes HW so Square runs on scalar)
 - cost tile staged ONCE per query in native [t][s][tau] layout; both DP
   directions read it with different strided APs (no 64B-packet shuffle)
 - DP runs both directions on 64 partitions, W in bf16, renorm via int16
   exponent tricks
"""
import sys
sys.path.insert(0, "/opt/trn_rl_repo")
import numpy as np
from contextlib import ExitStack

import concourse.bacc as bacc
import concourse.tile as tile
from concourse import mybir
from concourse.masks import make_identity

F32 = mybir.dt.float32
BF16 = mybir.dt.bfloat16
I32 = mybir.dt.int32
I16 = mybir.dt.int16
AF = mybir.ActivationFunctionType
ALU = mybir.AluOpType
LN2 = float(np.log(2.0))

NS, T, D = 25, 16, 2048
NQ_CORE = 64
G = 8                        # query groups of 128 rows (8 queries) each
NSTAU = NS * T               # 400
KCH = D // 128               # 16
SROWS = [128, 128, 128, 16]


def quake_rsqrt(nc, pool, x_f32, nrow, tag, final_scale=1.0):
    """y ~= final_scale / sqrt(x) on [nrow,1] f32 via fast-inverse-sqrt + 2
    Newton iterations (DVE only; avoids ACT table loads)."""
    y = pool.tile([128, 1], F32, tag=tag + "_y")
    t = pool.tile([128, 1], F32, tag=tag + "_t")
    yi = y.bitcast(I32)
    # yi = 0x5F3759DF - (xi >> 1)
    nc.vector.tensor_scalar(yi[:nrow], x_f32[:nrow].bitcast(I32), 1, None,
                            op0=ALU.logical_shift_right)
    nc.vector.tensor_scalar(yi[:nrow], yi[:nrow], 0x5F3759DF, -1,
                            op0=ALU.subtract, op1=ALU.mult)
    for it in range(2):
        # t = x*y*y ; t = 1.5 - 0.5*t ; y *= t
        nc.vector.tensor_tensor(t[:nrow], y[:nrow], y[:nrow], op=ALU.mult)
        nc.vector.tensor_tensor(t[:nrow], t[:nrow], x_f32[:nrow], op=ALU.mult)
        nc.vector.tensor_scalar(t[:nrow], t[:nrow], -0.5, 1.5,
                                op0=ALU.mult, op1=ALU.add)
        nc.vector.tensor_tensor(y[:nrow], y[:nrow], t[:nrow], op=ALU.mult)
    if final_scale != 1.0:
        nc.vector.tensor_scalar(y[:nrow], y[:nrow], float(final_scale), None,
                                op0=ALU.mult)
    return y


def build_core_kernel():
    nc = bacc.Bacc("TRN2", target_bir_lowering=False, debug=False)

    q_d = nc.dram_tensor("q", [NQ_CORE * T, D], F32, kind="ExternalInput").ap()
    s_d = nc.dram_tensor("s", [NSTAU, D], F32, kind="ExternalInput").ap()
    out_d = nc.dram_tensor("out", [NQ_CORE, 2 * NS], F32,
                           kind="ExternalOutput").ap()

    with tile.TileContext(nc) as tc, ExitStack() as ctx:
        const = ctx.enter_context(tc.tile_pool(name="const", bufs=1))
        eye_b = const.tile([128, 128], BF16, tag="eye_b")
        make_identity(nc, eye_b[:])
        bias_m10 = const.tile([128, 1], F32, tag="bias_m10")
        nc.vector.memset(bias_m10[:], -10.0)

        # normalized+transposed support: st_b[p=d%128][k=d//128][col=(s,tau)]
        stp = ctx.enter_context(tc.tile_pool(name="stp", bufs=1))
        st_b = stp.tile([128, KCH, NSTAU], BF16, tag="st_b")

        nsc = ctx.enter_context(tc.tile_pool(name="nsc", bufs=1))
        dmp = ctx.enter_context(tc.tile_pool(name="dmp", bufs=1))
        dump = dmp.tile([128, D], F32, tag="dump")

        # ---------------- S phase ----------------
        with tc.tile_pool(name="sraw", bufs=1) as sraw, \
             tc.tile_pool(name="spsum", bufs=2, space="PSUM") as spsum:
            snorm = []
            for i, nrow in enumerate(SROWS):
                t_ = sraw.tile([128, D], F32, tag=f"sraw{i}")
                nc.sync.dma_start(out=t_[:nrow],
                                  in_=s_d[128 * i:128 * i + nrow, :])
                n2 = nsc.tile([128, 1], F32, tag=f"sn2_{i}")
                nc.scalar.activation(dump[:nrow], t_[:nrow], AF.Square,
                                     accum_out=n2[:nrow])
                rs = quake_rsqrt(nc, nsc, n2, nrow, f"srs{i}")
                sb = sraw.tile([128, D], BF16, tag=f"sbf{i}")
                nc.vector.tensor_scalar(sb[:nrow], t_[:nrow], rs[:nrow], None,
                                        op0=ALU.mult)
                snorm.append((sb, nrow))
            for k in range(KCH):
                ps = spsum.tile([128, 512], BF16, tag="sps")
                for i, (sb, nrow) in enumerate(snorm):
                    nc.tensor.transpose(ps[:, 128 * i:128 * i + nrow],
                                        sb[:nrow, 128 * k:128 * (k + 1)],
                                        eye_b[:nrow, :nrow])
                nc.scalar.copy(st_b[:, k, :], ps[:, 0:NSTAU])

        # ---------------- Q phase: 8 groups of 128 rows ----------------
        # c_t[p=q (0:64)][t][s][tau] bf16 : cost exp(10*cos-10)
        cp = ctx.enter_context(tc.tile_pool(name="cp", bufs=1))
        c_t = cp.tile([64, T, NS, T], BF16, tag="c_t")

        qldp = ctx.enter_context(tc.tile_pool(name="qldp", bufs=2))
        qbfp = ctx.enter_context(tc.tile_pool(name="qbfp", bufs=2))
        qtp = ctx.enter_context(tc.tile_pool(name="qtp", bufs=2))
        t1p = ctx.enter_context(tc.tile_pool(name="t1p", bufs=2))
        ptr = ctx.enter_context(tc.tile_pool(name="ptr", bufs=2, space="PSUM"))
        pmm = ctx.enter_context(tc.tile_pool(name="pmm", bufs=2, space="PSUM"))

        for g in range(G):
            qraw = qldp.tile([128, D], F32, tag="qraw")
            nc.sync.dma_start(out=qraw[:], in_=q_d[128 * g:128 * (g + 1), :])
            # ||q||^2 per row -> scale 10/||q|| for the exp
            n2 = nsc.tile([128, 1], F32, tag="qn2")
            nc.scalar.activation(dump[:], qraw[:], AF.Square,
                                 accum_out=n2[:])
            rq10 = quake_rsqrt(nc, nsc, n2, 128, "qrs", final_scale=10.0)
            # cast to bf16 (gpsimd), transpose 16x 128x128 (PE), copy to SBUF
            qbf = qbfp.tile([128, D], BF16, tag="qbf")
            nc.gpsimd.tensor_copy(qbf[:], qraw[:])
            pt = ptr.tile([128, D], BF16, tag="pt")
            for k in range(KCH):
                nc.tensor.transpose(pt[:, 128 * k:128 * (k + 1)],
                                    qbf[:, 128 * k:128 * (k + 1)], eye_b[:])
            qt = qtp.tile([128, KCH, 128], BF16, tag="qt")
            nc.scalar.copy(qt[:], pt[:].rearrange("p (k f) -> p k f", k=KCH))
            # matmul: mm[qrow, (s,tau)] = sum_k qt[:,k].T @ st_b[:,k]
            mm = pmm.tile([128, NSTAU], F32, tag="mm")
            for k in range(KCH):
                nc.tensor.matmul(mm[:], qt[:, k, :], st_b[:, k, :],
                                 start=(k == 0), stop=(k == KCH - 1))
            # t1 = exp(10*cos - 10) in bf16
            t1 = t1p.tile([128, NSTAU], BF16, tag="t1")
            nc.scalar.activation(t1[:], mm[:], AF.Exp, bias=bias_m10[:],
                                 scale=rq10[:])
            # stage: 128 partitions (8 queries x 16 frames) -> 8 partitions
            nc.scalar.dma_start(
                out=c_t[8 * g:8 * (g + 1), :, :, :],
                in_=t1[:])

        # ---------------- DP phase (exp domain, both dirs per partition) ----
        dpp = ctx.enter_context(tc.tile_pool(name="dpp", bufs=1))
        w_t = dpp.tile([64, 2, NS, T + 1], BF16, tag="w_t")
        nc.vector.memset(w_t[:], 2.0)
        nc.vector.memset(w_t[:, :, :, 0:1], 1.0)
        o_t = dpp.tile([64, 2, NS], F32, tag="o_t")
        nc.vector.memset(o_t[:], 0.0)
        scratch = dpp.tile([64, 2, NS, T], BF16, tag="scratch")
        kmax = dpp.tile([64, 2, NS], BF16, tag="kmax")
        masked = dpp.tile([64, 2, NS], I16, tag="masked")
        krec = dpp.tile([64, 2, NS], I16, tag="krec")
        ef = dpp.tile([64, 2, NS], F32, tag="ef")
        otmp = dpp.tile([64, 2, NS], F32, tag="otmp")

        def renorm(a):
            wsl = w_t[:, :, :, a:T + 1]
            nc.vector.tensor_reduce(kmax[:], wsl, axis=mybir.AxisListType.X,
                                    op=ALU.max)
            nc.vector.tensor_scalar(masked[:], kmax[:].bitcast(I16),
                                    0x7F80, None, op0=ALU.bitwise_and)
            nc.vector.tensor_scalar(krec[:], masked[:], 0x7F00, -1,
                                    op0=ALU.subtract, op1=ALU.mult)
            nc.vector.tensor_copy(ef[:], masked[:])
            nc.vector.tensor_scalar(otmp[:], ef[:], LN2 / (1 << 7),
                                    -127.0 * LN2, op0=ALU.mult, op1=ALU.add)
            nc.vector.tensor_tensor(o_t[:], o_t[:], otmp[:], op=ALU.add)
            nc.vector.tensor_tensor(
                wsl, wsl,
                krec[:].bitcast(BF16).unsqueeze(-1)
                    .broadcast_to((64, 2, NS, T + 1 - a)),
                op=ALU.mult)

        for m in range(2, T + 3):           # m = 2..18
            j0 = max(1, m - 2)
            wm = (T + 1) - j0
            if m == T + 2:                  # last: dup, cost=1, l=T only
                nc.vector.scalar_tensor_tensor(
                    w_t[:, :, :, T:T + 1], w_t[:, :, :, T:T + 1], 2.0,
                    w_t[:, :, :, T - 1:T], op0=ALU.mult, op1=ALU.add)
                break
            wact = w_t[:, :, :, j0:T + 1]
            wsh = w_t[:, :, :, j0 - 1:T]
            tmp = scratch[:, :, :, 0:wm]
            if m == 2:
                nc.vector.scalar_tensor_tensor(tmp, wact, 2.0, wsh,
                                               op0=ALU.mult, op1=ALU.add)
            else:
                nc.vector.tensor_tensor(tmp, wact, wsh, op=ALU.add)
            # dir 0 ("b": rows l = support frame tau, cols m = query frame t)
            cb = c_t[:, m - 2, :, j0 - 1:j0 - 1 + wm]
            nc.vector.tensor_tensor(w_t[:, 0, :, j0:T + 1], tmp[:, 0], cb,
                                    op=ALU.mult)
            # dir 1 ("a": rows l = query frame t, cols m = support frame tau)
            ca = c_t[:, j0 - 1:j0 - 1 + wm, :, m - 2].rearrange(
                "p l s -> p s l")
            nc.vector.tensor_tensor(w_t[:, 1, :, j0:T + 1], tmp[:, 1], ca,
                                    op=ALU.mult)
            if m in (5, 9, 13, 16):
                renorm(m - 2)

        lnw = dpp.tile([64, 2 * NS], F32, tag="lnw")
        nc.scalar.activation(lnw[:], w_t[:, :, :, T], AF.Ln)
        ans = dpp.tile([64, 2 * NS], F32, tag="ans")
        nc.vector.tensor_tensor(ans[:], lnw[:],
                                o_t[:].rearrange("p a b -> p (a b)"),
                                op=ALU.add)
        nc.sync.dma_start(out=out_d, in_=ans[:])

    nc.compile()
    return nc


_NC_CACHE = {}


def _get_nc():
    if "nc" not in _NC_CACHE:
        _NC_CACHE["nc"] = build_core_kernel()
    return _NC_CACHE["nc"]


def kernel(support_features, target_features, support_labels):
    out, _ = host_kernel(support_features, target_features, support_labels,
                         nc=_get_nc())
    return out


def host_kernel(support_features, target_features, support_labels, nc=None,
                run_hw=True, trace=False):
    n_support, T_, d = support_features.shape
    nq = target_features.shape[0]
    assert (n_support, T_, d) == (NS, T, D) and nq == 512
    if nc is None:
        nc = build_core_kernel()
    s_flat = np.ascontiguousarray(support_features.reshape(NSTAU, D))
    in_maps = []
    for c in range(8):
        qs = target_features[64 * c:64 * (c + 1)].reshape(NQ_CORE * T, D)
        in_maps.append({"q": np.ascontiguousarray(qs), "s": s_flat})
    from concourse.bass_utils import run_bass_kernel_spmd
    res = run_bass_kernel_spmd(nc, in_maps, list(range(8)), trace=trace)
    outs = [np.asarray(r["out"]) for r in res.results]
    # out[q, dir*NS + s]; dists = -0.1 * (dir0 + dir1)
    dists = np.concatenate(
        [-0.1 * (o[:, 0:NS] + o[:, NS:2 * NS]) for o in outs], axis=0)
    onehot = (np.asarray(support_labels)[:, None]
              == np.arange(5)[None, :]).astype(np.float32)
    class_dists = (dists.astype(np.float32) @ onehot) / onehot.sum(axis=0)
    return class_dists.astype(np.float32), res


# revision 9
# speedup vs baseline: 2.3021x; 1.2254x over previous
"""OTAM (5-way 5-shot video few-shot) kernel for Trainium2, 8 NeuronCores.

Self-contained: kernel(**inputs) takes full inputs, shards 512 queries over
8 cores (64 each), runs a Bass/Tile kernel per core, gathers class means.

v2 design (v0 baseline 328us, v1 175us):
 - bf16 matmuls + transposes
 - q and s normalized+cast to bf16 in ONE DVE op; exp scale is constant
   (shorter cross-engine dependency chain; no gpsimd cast)
 - norms via ACT Square+accum; rsqrt via 1-Newton-iteration Quake on DVE
 - cost tile staged twice (both DP dirs) in native [t][s][tau] layout via
   two 800B-packet DMAs per group; DP on all 128 partitions
 - DP in bf16, 2 renorms (int16 exponent tricks)
"""
import sys
sys.path.insert(0, "/opt/trn_rl_repo")
import numpy as np
from contextlib import ExitStack

import concourse.bacc as bacc
import concourse.tile as tile
from concourse import mybir
from concourse.masks import make_identity

F32 = mybir.dt.float32
BF16 = mybir.dt.bfloat16
I32 = mybir.dt.int32
I16 = mybir.dt.int16
AF = mybir.ActivationFunctionType
ALU = mybir.AluOpType
LN2 = float(np.log(2.0))

NS, T, D = 25, 16, 2048
NQ_CORE = 64
G = 8                        # query groups of 128 rows (8 queries) each
NSTAU = NS * T               # 400
KCH = D // 128               # 16
SROWS = [128, 128, 128, 16]


def quake_rsqrt(nc, pool, x_f32, nrow, tag, iters=1):
    """y ~= 1/sqrt(x) on [nrow,1] f32 (DVE only; ~0.2% rel err at 1 iter)."""
    y = pool.tile([128, 1], F32, tag=tag + "_y")
    t = pool.tile([128, 1], F32, tag=tag + "_t")
    yi = y.bitcast(I32)
    nc.vector.tensor_scalar(yi[:nrow], x_f32[:nrow].bitcast(I32), 1, None,
                            op0=ALU.logical_shift_right)
    nc.vector.tensor_scalar(yi[:nrow], yi[:nrow], 0x5F3759DF, -1,
                            op0=ALU.subtract, op1=ALU.mult)
    for it in range(iters):
        nc.vector.tensor_tensor(t[:nrow], y[:nrow], y[:nrow], op=ALU.mult)
        nc.vector.tensor_tensor(t[:nrow], t[:nrow], x_f32[:nrow], op=ALU.mult)
        nc.vector.tensor_scalar(t[:nrow], t[:nrow], -0.5, 1.5,
                                op0=ALU.mult, op1=ALU.add)
        nc.vector.tensor_tensor(y[:nrow], y[:nrow], t[:nrow], op=ALU.mult)
    return y


def build_core_kernel():
    nc = bacc.Bacc("TRN2", target_bir_lowering=False, debug=False)

    q_d = nc.dram_tensor("q", [NQ_CORE * T, D], F32, kind="ExternalInput").ap()
    s_d = nc.dram_tensor("s", [NSTAU, D], F32, kind="ExternalInput").ap()
    out_d = nc.dram_tensor("out", [128, NS], F32, kind="ExternalOutput").ap()

    with tile.TileContext(nc) as tc, ExitStack() as ctx:
        const = ctx.enter_context(tc.tile_pool(name="const", bufs=1))
        eye_b = const.tile([128, 128], BF16, tag="eye_b")
        make_identity(nc, eye_b[:])
        bias_m10 = const.tile([128, 1], F32, tag="bias_m10")
        nc.vector.memset(bias_m10[:], -10.0)

        # normalized+transposed support: st_b[p=d%128][k=d//128][col=(s,tau)]
        stp = ctx.enter_context(tc.tile_pool(name="stp", bufs=1))
        st_b = stp.tile([128, KCH, NSTAU], BF16, tag="st_b")

        nsc = ctx.enter_context(tc.tile_pool(name="nsc", bufs=1))
        dmp = ctx.enter_context(tc.tile_pool(name="dmp", bufs=1))
        dump = dmp.tile([128, D], F32, tag="dump")

        # ---------------- S phase ----------------
        with tc.tile_pool(name="sraw", bufs=1) as sraw, \
             tc.tile_pool(name="spsum", bufs=2, space="PSUM") as spsum:
            snorm = []
            for i, nrow in enumerate(SROWS):
                t_ = sraw.tile([128, D], F32, tag=f"sraw{i}")
                nc.sync.dma_start(out=t_[:nrow],
                                  in_=s_d[128 * i:128 * i + nrow, :])
                n2 = nsc.tile([128, 1], F32, tag=f"sn2_{i}")
                nc.scalar.activation(dump[:nrow], t_[:nrow], AF.Square,
                                     accum_out=n2[:nrow])
                rs = quake_rsqrt(nc, nsc, n2, nrow, f"srs{i}")
                sb = sraw.tile([128, D], BF16, tag=f"sbf{i}")
                nc.vector.tensor_scalar(sb[:nrow], t_[:nrow], rs[:nrow], None,
                                        op0=ALU.mult)
                snorm.append((sb, nrow))
            for k in range(KCH):
                ps = spsum.tile([128, 512], BF16, tag="sps")
                for i, (sb, nrow) in enumerate(snorm):
                    nc.tensor.transpose(ps[:, 128 * i:128 * i + nrow],
                                        sb[:nrow, 128 * k:128 * (k + 1)],
                                        eye_b[:nrow, :nrow])
                nc.scalar.copy(st_b[:, k, :], ps[:, 0:NSTAU])

        # ---------------- Q phase: 8 groups of 128 rows ----------------
        # c_t[p][t][s][tau] bf16: partitions q and 64+q hold query q's costs
        cp = ctx.enter_context(tc.tile_pool(name="cp", bufs=1))
        c_t = cp.tile([128, T, NS, T], BF16, tag="c_t")

        qldp = ctx.enter_context(tc.tile_pool(name="qldp", bufs=2))
        qbfp = ctx.enter_context(tc.tile_pool(name="qbfp", bufs=2))
        qtp = ctx.enter_context(tc.tile_pool(name="qtp", bufs=2))
        t1p = ctx.enter_context(tc.tile_pool(name="t1p", bufs=2))
        ptr = ctx.enter_context(tc.tile_pool(name="ptr", bufs=2, space="PSUM"))
        pmm = ctx.enter_context(tc.tile_pool(name="pmm", bufs=2, space="PSUM"))

        for g in range(G):
            qraw = qldp.tile([128, D], F32, tag="qraw")
            nc.sync.dma_start(out=qraw[:], in_=q_d[128 * g:128 * (g + 1), :])
            n2 = nsc.tile([128, 1], F32, tag="qn2")
            nc.scalar.activation(dump[:], qraw[:], AF.Square,
                                 accum_out=n2[:])
            rq = quake_rsqrt(nc, nsc, n2, 128, "qrs")
            # normalize + cast to bf16 in one DVE op
            qbf = qbfp.tile([128, D], BF16, tag="qbf")
            nc.vector.tensor_scalar(qbf[:], qraw[:], rq[:], None, op0=ALU.mult)
            pt = ptr.tile([128, D], BF16, tag="pt")
            for k in range(KCH):
                nc.tensor.transpose(pt[:, 128 * k:128 * (k + 1)],
                                    qbf[:, 128 * k:128 * (k + 1)], eye_b[:])
            qt = qtp.tile([128, KCH, 128], BF16, tag="qt")
            nc.scalar.copy(qt[:], pt[:].rearrange("p (k f) -> p k f", k=KCH))
            mm = pmm.tile([128, NSTAU], F32, tag="mm")
            for k in range(KCH):
                nc.tensor.matmul(mm[:], qt[:, k, :], st_b[:, k, :],
                                 start=(k == 0), stop=(k == KCH - 1))
            # t1 = exp(10*cos - 10) in bf16 (constant scale/bias)
            t1 = t1p.tile([128, NSTAU], BF16, tag="t1")
            nc.scalar.activation(t1[:], mm[:], AF.Exp, bias=bias_m10[:],
                                 scale=10.0)
            # stage both DP copies: [128,400] -> 8 partitions each
            nc.scalar.dma_start(out=c_t[8 * g:8 * (g + 1), :, :, :], in_=t1[:])
            nc.scalar.dma_start(out=c_t[64 + 8 * g:64 + 8 * (g + 1), :, :, :],
                                in_=t1[:])

        # ---------------- DP phase (exp domain) ----------------
        # partition q: dir "b" (rows l = support frame tau, cols = t)
        # partition 64+q: dir "a" (rows l = query frame t, cols = tau)
        dpp = ctx.enter_context(tc.tile_pool(name="dpp", bufs=1))
        w_t = dpp.tile([128, NS, T + 1], BF16, tag="w_t")
        nc.vector.memset(w_t[:], 2.0)
        nc.vector.memset(w_t[:, :, 0:1], 1.0)
        o_t = dpp.tile([128, NS], F32, tag="o_t")
        nc.vector.memset(o_t[:], 0.0)
        scratch = dpp.tile([128, NS, T], BF16, tag="scratch")
        kmax = dpp.tile([128, NS], BF16, tag="kmax")
        masked = dpp.tile([128, NS], I16, tag="masked")
        krec = dpp.tile([128, NS], I16, tag="krec")
        ef = dpp.tile([128, NS], F32, tag="ef")
        otmp = dpp.tile([128, NS], F32, tag="otmp")

        def renorm(a):
            wsl = w_t[:, :, a:T + 1]
            nc.vector.tensor_reduce(kmax[:], wsl, axis=mybir.AxisListType.X,
                                    op=ALU.max)
            nc.vector.tensor_scalar(masked[:], kmax[:].bitcast(I16),
                                    0x7F80, None, op0=ALU.bitwise_and)
            nc.vector.tensor_scalar(krec[:], masked[:], 0x7F00, -1,
                                    op0=ALU.subtract, op1=ALU.mult)
            nc.vector.tensor_copy(ef[:], masked[:])
            nc.vector.tensor_scalar(otmp[:], ef[:], LN2 / (1 << 7),
                                    -127.0 * LN2, op0=ALU.mult, op1=ALU.add)
            nc.vector.tensor_tensor(o_t[:], o_t[:], otmp[:], op=ALU.add)
            nc.vector.tensor_tensor(
                wsl, wsl,
                krec[:].bitcast(BF16).unsqueeze(-1)
                    .broadcast_to((128, NS, T + 1 - a)),
                op=ALU.mult)

        for m in range(2, T + 3):           # m = 2..18
            j0 = max(1, m - 2)
            wm = (T + 1) - j0
            if m == T + 2:                  # last: dup, cost=1, l=T only
                nc.vector.scalar_tensor_tensor(
                    w_t[:, :, T:T + 1], w_t[:, :, T:T + 1], 2.0,
                    w_t[:, :, T - 1:T], op0=ALU.mult, op1=ALU.add)
                break
            wact = w_t[:, :, j0:T + 1]
            wsh = w_t[:, :, j0 - 1:T]
            tmp = scratch[:, :, 0:wm]
            if m == 2:
                nc.vector.scalar_tensor_tensor(tmp, wact, 2.0, wsh,
                                               op0=ALU.mult, op1=ALU.add)
            else:
                nc.vector.tensor_tensor(tmp, wact, wsh, op=ALU.add)
            # dir b on partitions 0:64 (l = tau contiguous)
            cb = c_t[0:64, m - 2, :, j0 - 1:j0 - 1 + wm]
            nc.vector.tensor_tensor(w_t[0:64, :, j0:T + 1], tmp[0:64], cb,
                                    op=ALU.mult)
            # dir a on partitions 64:128 (l = t strided)
            ca = c_t[64:128, j0 - 1:j0 - 1 + wm, :, m - 2].rearrange(
                "p l s -> p s l")
            nc.vector.tensor_tensor(w_t[64:128, :, j0:T + 1], tmp[64:128], ca,
                                    op=ALU.mult)
            if m in (9, 16):
                renorm(m - 2)

        lnw = dpp.tile([128, NS], F32, tag="lnw")
        nc.scalar.activation(lnw[:], w_t[:, :, T], AF.Ln)
        ans = dpp.tile([128, NS], F32, tag="ans")
        nc.vector.tensor_tensor(ans[:], lnw[:], o_t[:], op=ALU.add)
        nc.sync.dma_start(out=out_d, in_=ans[:])

    nc.compile()
    return nc


_NC_CACHE = {}


def _get_nc():
    if "nc" not in _NC_CACHE:
        _NC_CACHE["nc"] = build_core_kernel()
    return _NC_CACHE["nc"]


def kernel(support_features, target_features, support_labels):
    out, _ = host_kernel(support_features, target_features, support_labels,
                         nc=_get_nc())
    return out


def host_kernel(support_features, target_features, support_labels, nc=None,
                run_hw=True, trace=False):
    n_support, T_, d = support_features.shape
    nq = target_features.shape[0]
    assert (n_support, T_, d) == (NS, T, D) and nq == 512
    if nc is None:
        nc = build_core_kernel()
    s_flat = np.ascontiguousarray(support_features.reshape(NSTAU, D))
    in_maps = []
    for c in range(8):
        qs = target_features[64 * c:64 * (c + 1)].reshape(NQ_CORE * T, D)
        in_maps.append({"q": np.ascontiguousarray(qs), "s": s_flat})
    from concourse.bass_utils import run_bass_kernel_spmd
    res = run_bass_kernel_spmd(nc, in_maps, list(range(8)), trace=trace)
    outs = [np.asarray(r["out"]) for r in res.results]
    # partition q = dir b, partition 64+q = dir a; dists = -0.1*(a+b)
    dists = np.concatenate(
        [-0.1 * (o[0:64] + o[64:128]) for o in outs], axis=0)
    onehot = (np.asarray(support_labels)[:, None]
              == np.arange(5)[None, :]).astype(np.float32)
    class_dists = (dists.astype(np.float32) @ onehot) / onehot.sum(axis=0)
    return class_dists.astype(np.float32), res
